# revision 10
# baseline (speedup 1.0000x reference)
"""Causal multi-head self-attention on 8 TRN2 NeuronCores (Bass/Tile).

Problem: z[B=2,T=2048,D=1024], per-head dim 64, H=16 heads, fp32.
Sharding: core = b*4 + g  (b = batch, g = head-group of 4 heads).
Each core computes, for its batch b and heads 4g..4g+3:
    Q.T/K.T = (Wq/Wk slice).T-projection of z.T   [256, 2048] (head-stacked)
    V       = z @ Wv slice                         [2048, 256] (+ ones column)
    S.T     = K.T-slices vs Q.T  (partition = key j, free = query i)
    P       = exp(S/8) * causal mask   (no max-subtraction needed: |S|≲8σ)
    ctx.T   = [V | 1].T @ P   -> row 64 holds the softmax denominators
    out.T  += Wo-rows.T @ (ctx.T / denom)          [1024, 2048] partial
Host sums the 4 per-batch partials and transposes back.

All matmuls run as float32r (full-rate fp32 path; measured ~1.4e-4 relmax
vs fp64 on this HW, identical to plain fp32 output in the probe).
"""
import sys
import types

import numpy as np

# ── antenv.axon_hooks shim (NTFF profiling; agent image lacks the module) ──
import antenv  # noqa: F401

if "antenv.axon_hooks" not in sys.modules:
    _hooks = types.ModuleType("antenv.axon_hooks")
    _HOOK = [None]
    _hooks.set_axon_ntff_profile_hook = lambda h: _HOOK.__setitem__(0, h)
    _hooks.get_axon_ntff_profile_hook = lambda: _HOOK[0]
    sys.modules["antenv.axon_hooks"] = _hooks
    antenv.axon_hooks = _hooks
    try:
        from trn_agent_boot.trn_boot import _ntff_profile_via_ctypes

        _hooks.set_axon_ntff_profile_hook(
            _ntff_profile_via_ctypes("/opt/axon/libaxon_pjrt.so")
        )
    except Exception:
        pass

import concourse.bass as bass  # noqa: E402
import concourse.tile as tile  # noqa: E402
import concourse.mybir as mybir  # noqa: E402
import concourse.bass_utils as bass_utils  # noqa: E402
from bass_rust import ScopedClock  # noqa: E402

bass_utils.upload_artifacts = lambda tmpdir: ""

F32 = mybir.dt.float32
F32R = mybir.dt.float32r
EXP = mybir.ActivationFunctionType.Exp
LN = mybir.ActivationFunctionType.Ln

# ── workaround: this walrus build allows max ONE sync-wait per instruction ──
_wsplit = [0]


def _split_excess_waits(nc, limit=1):
    n = 0
    for fn in nc.m.functions:
        for blk in fn.blocks:
            out = []
            for inst in blk.instructions:
                si = inst.sync_info
                if si is not None and len(si.on_wait) > limit:
                    ws = list(si.on_wait)
                    keep = ws[-limit:]
                    hoist = ws[:-limit]
                    for i in range(0, len(hoist), limit):
                        _wsplit[0] += 1
                        out.append(
                            mybir.InstNoOp(
                                name=f"I-wsplit-{_wsplit[0]}",
                                engine=inst.engine,
                                sync_info=mybir.SyncInfo(
                                    on_wait=hoist[i : i + limit], on_update=[]
                                ),
                                bass_nofuse=True,
                            )
                        )
                        n += 1
                    si.on_wait = keep
                out.append(inst)
            blk.instructions = out
    return n


def _patched_drain_and_barrier(self, tick_clock, wait_clock):
    nc = self.nc
    drain_inst = nc.sync.drain()
    wait_clock.add_sem_waits(
        drain_inst.ins, ScopedClock({None: tick_clock.global_clock})
    )
    si = drain_inst.ins.sync_info
    if si is not None and len(si.on_wait) > 1:
        waits = list(si.on_wait)
        si.on_wait = waits[:1]
        for w in waits[1:]:
            d2 = nc.sync.drain()
            d2.ins.sync_info = mybir.SyncInfo(on_wait=[w], on_update=[])
    nc.all_engine_barrier()
    assert self.sems is not None
    popped = nc._tile_sem_poison_stack.pop()
    assert popped is self._sem_poison
    nc.clear_and_free_semaphores(list(self.sems.allocated().values()))
    nc.all_engine_barrier()


tile.TileContext._drain_and_barrier = _patched_drain_and_barrier

# ── problem shape (hardcoded) ──
B, T, D, H, HD = 2, 2048, 1024, 16, 64
HPC = 4  # heads per core
DG = HPC * HD  # 256 projection cols per core
NQ = 512  # query-chunk width (one PSUM bank of fp32)
KT = T // 128  # 16 key tiles
NCH = T // NQ  # 4 query chunks
D8 = D // 128  # 8 contraction tiles
SCALE = 1.0 / np.sqrt(HD)


def build_kernel():
    nc = bass.Bass("TRN2", target_bir_lowering=False, debug=False)
    zt_d = nc.dram_tensor("zt", [D, T], F32R, kind="ExternalInput").ap()
    wq_d = nc.dram_tensor("wq", [D, DG], F32R, kind="ExternalInput").ap()
    wk_d = nc.dram_tensor("wk", [D, DG], F32R, kind="ExternalInput").ap()
    wv_d = nc.dram_tensor("wv", [D, DG], F32R, kind="ExternalInput").ap()
    wo_d = nc.dram_tensor("wo", [DG, D], F32R, kind="ExternalInput").ap()
    mk_d = nc.dram_tensor("mk", [4, 128, NQ], F32R, kind="ExternalInput").ap()
    on_d = nc.dram_tensor("on", [128, KT * HPC], F32R, kind="ExternalInput").ap()
    ot_d = nc.dram_tensor("ot", [D, T], F32, kind="ExternalOutput").ap()

    with tile.TileContext(nc) as tc:
        with (
            tc.tile_pool(name="persist", bufs=1) as persist,
            tc.tile_pool(name="ps_proj", bufs=2, space="PSUM") as ps_proj,
            tc.tile_pool(name="ps_scores", bufs=2, space="PSUM") as ps_scores,
            tc.tile_pool(name="ps_ctx", bufs=2, space="PSUM") as ps_ctx,
        ):
            wq_t = persist.tile([128, D8, DG], F32R)
            wk_t = persist.tile([128, D8, DG], F32R)
            wv_t = persist.tile([128, D8, DG], F32R)
            wo_t = persist.tile([128, DG // 128, D], F32R)
            mk_t = persist.tile([128, 4, NQ], F32R)
            # head-pair stacked Q.T / K.T: partitions 0-63 head 2p, 64-127 head 2p+1
            qt_t = [persist.tile([128, T], F32R, tag=f"qt{p}", name=f"qt{p}") for p in range(2)]
            kt_t = [persist.tile([128, T], F32R, tag=f"kt{p}", name=f"kt{p}") for p in range(2)]
            # V in natural layout per (key-tile, head): 64 cols + ones col
            v_t = persist.tile([128, KT, HPC, HD + 1], F32R)
            # normalized ctx.T, stacked like qt (kk=0: heads 0,1; kk=1: heads 2,3)
            ct_t = [persist.tile([128, T], F32R, tag=f"ct{p}", name=f"ct{p}") for p in range(2)]

            nc.sync.dma_start(wq_t[:], wq_d.rearrange("(a p) c -> p a c", p=128))
            nc.sync.dma_start(wk_t[:], wk_d.rearrange("(a p) c -> p a c", p=128))
            nc.sync.dma_start(wv_t[:], wv_d.rearrange("(a p) c -> p a c", p=128))
            nc.sync.dma_start(wo_t[:], wo_d.rearrange("(a p) c -> p a c", p=128))
            nc.sync.dma_start(mk_t[:], mk_d.rearrange("a p c -> p a c"))
            # ones: memset can't write f32r on this ISA -> DMA from DRAM
            nc.sync.dma_start(
                v_t[:, :, :, HD],
                on_d.rearrange("p (a b) -> p a b", a=KT),
            )
            ones1 = persist.tile([1, HD], F32R)
            nc.sync.dma_start(ones1[:], on_d[0:1, 0:HD])

            with tc.tile_pool(name="ztp", bufs=1) as ztp:
                zt_t = ztp.tile([128, D8, T], F32R)
                for k8 in range(D8):
                    nc.sync.dma_start(
                        zt_t[:, k8, :], zt_d[k8 * 128 : (k8 + 1) * 128, :]
                    )

                # ── Q.T and K.T projections: out [qkv-col, token] ──
                for w_t, dst in ((wq_t, qt_t), (wk_t, kt_t)):
                    for m in range(2):
                        for c in range(NCH):
                            ps = ps_proj.tile([128, NQ], F32, tag="proj", name="proj_ps")
                            for k8 in range(D8):
                                nc.tensor.matmul(
                                    ps[:],
                                    w_t[:, k8, m * 128 : (m + 1) * 128],
                                    zt_t[:, k8, c * NQ : (c + 1) * NQ],
                                    start=(k8 == 0),
                                    stop=(k8 == D8 - 1),
                                )
                            nc.scalar.copy(
                                dst[m][:, c * NQ : (c + 1) * NQ], ps[:]
                            )

                # ── V projection: out [token, qkv-col] ──
                for vm in range(KT):
                    ps = ps_proj.tile([128, NQ], F32, tag="proj", name="proj_ps")
                    for k8 in range(D8):
                        nc.tensor.matmul(
                            ps[:, 0:DG],
                            zt_t[:, k8, vm * 128 : (vm + 1) * 128],
                            wv_t[:, k8, :],
                            start=(k8 == 0),
                            stop=(k8 == D8 - 1),
                        )
                    nc.scalar.copy(
                        v_t[:, vm, :, 0:HD],
                        ps[:, 0:DG].rearrange("p (h d) -> p h d", h=HPC),
                    )

            # ── attention, one head-pair at a time ──
            with (
                tc.tile_pool(name="pbuf", bufs=3) as pbuf,
                tc.tile_pool(name="nrm", bufs=2) as nrm,
                tc.tile_pool(name="stg", bufs=2) as stg,
            ):
                for p in range(2):
                    for c in range(NCH):
                        nbatch = (4 * c + 4) // 2
                        ctxs = []
                        for h in range(2):
                            ctxs.append(ps_ctx.tile([128, NQ], F32, tag="ctx", name="ctx"))
                        for b in range(nbatch):
                            for h in range(2):
                                hb = 64 * h
                                s_ps = ps_scores.tile([128, 2, NQ], F32, tag="s", name="s_ps")
                                for j in range(2):
                                    kt = 2 * b + j
                                    nc.tensor.matmul(
                                        s_ps[:, j, :],
                                        kt_t[p][
                                            hb : hb + 64,
                                            kt * 128 : (kt + 1) * 128,
                                        ],
                                        qt_t[p][
                                            hb : hb + 64, c * NQ : (c + 1) * NQ
                                        ],
                                        start=True,
                                        stop=True,
                                    )
                                p_t = pbuf.tile([128, 2, NQ], F32R, tag="p", name="p_t")
                                nc.scalar.activation(
                                    out=p_t[:], in_=s_ps[:], func=EXP,
                                    scale=float(SCALE),
                                )
                                for j in range(2):
                                    kt = 2 * b + j
                                    d_idx = kt - 4 * c
                                    if d_idx >= 0:
                                        nc.vector.tensor_mul(
                                            p_t[:, j, :],
                                            p_t[:, j, :],
                                            mk_t[:, d_idx, :],
                                        )
                                for j in range(2):
                                    kt = 2 * b + j
                                    nc.tensor.matmul(
                                        ctxs[h][0 : HD + 1, :],
                                        v_t[:, kt, 2 * p + h, :],
                                        p_t[:, j, :],
                                        start=(kt == 0),
                                        stop=(kt == 4 * c + 3),
                                    )
                        # normalize: ctx rows /= sums row (row 64).
                        # recip = exp(-ln(s)) on ACT (custom-DVE recip ops
                        # don't codegen on this walrus); broadcast along
                        # partitions via a K=1 ones-matmul.
                        for h in range(2):
                            ln_t = nrm.tile([1, NQ], F32, tag="ln", name="ln_t")
                            r_t = nrm.tile([1, NQ], F32R, tag="r", name="r_t")
                            rb_t = nrm.tile([128, NQ], F32, tag="rb", name="rb_t")
                            nc.scalar.activation(
                                out=ln_t[:], in_=ctxs[h][HD : HD + 1, :],
                                func=LN,
                            )
                            nc.scalar.activation(
                                out=r_t[:], in_=ln_t[:], func=EXP, scale=-1.0
                            )
                            bc_ps = ps_proj.tile(
                                [128, NQ], F32, tag="proj", name="proj_ps"
                            )
                            nc.tensor.matmul(
                                bc_ps[0:HD, :],
                                ones1[0:1, :],
                                r_t[0:1, :],
                                start=True,
                                stop=True,
                            )
                            nc.vector.tensor_copy(rb_t[0:HD, :], bc_ps[0:HD, :])
                            nc.vector.tensor_mul(
                                ct_t[p][
                                    h * 64 : h * 64 + HD,
                                    c * NQ : (c + 1) * NQ,
                                ],
                                ctxs[h][0:HD, :],
                                rb_t[0:HD, :],
                            )

                # ── output projection: out.T [outcol, token] ──
                for mo in range(D // 128):
                    st = stg.tile([128, T], F32, tag="st", name="st")
                    for c in range(NCH):
                        ps = ps_proj.tile([128, NQ], F32, tag="proj", name="proj_ps")
                        for kk in range(2):
                            nc.tensor.matmul(
                                ps[:],
                                wo_t[:, kk, mo * 128 : (mo + 1) * 128],
                                ct_t[kk][:, c * NQ : (c + 1) * NQ],
                                start=(kk == 0),
                                stop=(kk == 1),
                            )
                        nc.vector.tensor_copy(
                            st[:, c * NQ : (c + 1) * NQ], ps[:]
                        )
                    nc.sync.dma_start(
                        ot_d[mo * 128 : (mo + 1) * 128, :], st[:]
                    )

    return nc


def _host_inputs(z, w_q, w_k, w_v, w_o):
    """Per-core input maps (host-side sharding + transposes)."""
    z = np.asarray(z, dtype=np.float32)
    w_q = np.asarray(w_q, dtype=np.float32)
    w_k = np.asarray(w_k, dtype=np.float32)
    w_v = np.asarray(w_v, dtype=np.float32)
    w_o = np.asarray(w_o, dtype=np.float32)

    pj = np.arange(128)[:, None]
    fi = np.arange(NQ)[None, :]
    masks = np.stack(
        [(fi >= pj + 128 * d).astype(np.float32) for d in range(4)]
    )  # [4, 128, NQ]

    zt = [np.ascontiguousarray(z[b].T) for b in range(B)]
    in_maps = []
    for core in range(8):
        b, g = core // 4, core % 4
        cs = slice(g * DG, (g + 1) * DG)
        in_maps.append(
            {
                "zt": zt[b],
                "wq": np.ascontiguousarray(w_q[:, cs]),
                "wk": np.ascontiguousarray(w_k[:, cs]),
                "wv": np.ascontiguousarray(w_v[:, cs]),
                "wo": np.ascontiguousarray(w_o[cs, :]),
                "mk": masks,
                "on": np.ones((128, KT * HPC), dtype=np.float32),
            }
        )
    return in_maps


def run(z, w_q, w_k, w_v, w_o, trace=False, trace_cores=None):
    """Build + run on 8 cores; returns (output [B,T,D], BassKernelResults)."""
    nc = build_kernel()
    n = _split_excess_waits(nc)
    if n:
        print(f"[kernel] split {n} excess sync-waits onto nops", file=sys.stderr)
    in_maps = _host_inputs(z, w_q, w_k, w_v, w_o)
    res = bass_utils.run_bass_kernel_spmd(
        nc, in_maps, list(range(8)), trace=trace, trace_cores=trace_cores
    )
    out = np.zeros((B, T, D), dtype=np.float64)
    for core in range(8):
        out[core // 4] += res.results[core]["ot"].T.astype(np.float64)
    return out.astype(np.float32), res


def kernel(z, w_q, w_k, w_v, w_o):
    out, _ = run(z, w_q, w_k, w_v, w_o, trace=False)
    return out
